# revision 3
# baseline (speedup 1.0000x reference)
"""Trainium2 Bass kernel for DepthLossForImgBEV (weighted one-hot depth BCE).

Math: with x = raw logits (B,N,D,H,W), gt = depth_gt (B,N,H,W):
  loss = 3.0 * [ sum_{valid px, d} softplus(x) - sum_{valid px} x[idx] ] / numel
and the device computes sum softplus via the sigmoid-product identity
  sum_d softplus(x_d) = -ln prod_d sigmoid(-x_d),
so the only transcendental pass is ONE sigmoid over the logits; the
depth-bin reduction becomes a pure multiply chain and the final ln is a
1-instruction bit-trick on DVE. The ACT engine never switches tables.

Device pipeline per core (shard = 8 of 64 H-rows; partitions = 16 depth
bins x 8 h-rows; free = 7 t-tiles x (12 cameras x 176 w)):
  - input xn = fp8_e3m4(-x), host-packed [128, 7*2112] fully contiguous
    per partition; invalid pixels baked to +15.5 (sigmoid -> 1.0 in bf16),
    which removes the mask tensor and mask-add pass entirely.
  - ACT: sigmoid(xn) per chunk, fp8-in -> bf16-out (4 instrs, one
    ACT_TABLE_LOAD for the whole NEFF).
  - DVE: t = prod_t sigmoid(-x_t): 5 in-place bf16 muls (2-byte 2x mode)
    + 1 f32-out mul.
  - DVE: ln(t) ~= (bitcast_i32(t) * ln2/2^23) - 127*ln2 + kappa*ln2, one
    scalar_tensor_tensor with accum_out. kappa = mean(log2(1+f)-f) zero-
    means the mantissa-interp error for log-uniform mantissae; masked
    columns (t == 1.0 exactly) each read +kappa*ln2, which the host
    subtracts exactly (count = 16 * #invalid pixels).
Host: a_total = -(sum(out) - n_masked_cols*kappa*ln2); one-hot gather term
b_total by fancy-indexing exact f32 logits (0.4% of FLOPs); loss =
3*(a_total - b_total)/numel.

Measured on trn2 (8 cores, axon), interleaved same-round reps-slope
(reps=600 vs 1), median over rounds:
  - previous kernel (exp + pair-fold + ln):  25.3 us/pass
  - this kernel:                             11.4 us/pass  (2.2x)
Same-window ablations: DMA floor 3.7 us, sigmoid-only 15.3/16.3 of full,
+muls +0.4 us, +lnapprox +0.6 us -> ACT-sigmoid-bound; DVE and DMA fully
hidden. Engine rates calibrated on HW: ACT ~0.86-1.03 ns/el (fp8 input
fastest), DVE bf16 mul 0.46 (2x), stt 0.98 (no 2x), DMA ~0.27-0.35 ns/B.
Rel err vs fp32 jax reference: 5.7e-05 on HW (numpy bit-exact sim of the
pipeline predicts 5.8e-05; fp8-e3m4 quantization and bf16 product noise
are zero-mean and cancel over the 12.6M-element sum).

Notes:
  - walrus core_v2/v3 codegen accepts only ONE fused sem wait per
    instruction on this toolchain -> _split_excess_waits hoists extras
    into standalone EventSemaphore instructions.
  - softplus is not in this compiler's act tables (the set exists but maps
    no Softplus function); sigmoid+ln share no table set, hence the DVE
    bit-trick ln instead of ACT Ln.
"""
import numpy as np

B, N, D, H, W = 2, 6, 112, 64, 176
M = 8  # cores
HSH = H // M  # 8 h-rows per core
DD = 16  # depth bins per partition block
NT = D // DD  # 7 t-tiles
BN = B * N  # 12
P = 128
FREE = BN * W  # 2112
NUMEL = B * N * D * H * W
MASK_VAL = 15.5  # sigmoid(15.5) = 1 - 1.9e-7 -> rounds to 1.0 in bf16

LN2 = 0.6931471805599453
# kappa (bits): zero-means f - log2(1+f) over f~U[0,1]
KAPPA = (2.0 * LN2 - 1.0) / LN2 - 0.5  # 0.057304959...
LN_C = LN2 / (1 << 23)
LN_D = (-127.0 + KAPPA) * LN2

_CACHE = {}


def _build_bass(reps=1, dma_only=False, drop_dve=False, drop_ln=False,
                xbufs=5, ebufs=4):
    from contextlib import ExitStack

    import concourse.bass as bass
    import concourse.mybir as mybir
    import concourse.tile as tile

    f32 = mybir.dt.float32
    bf16 = mybir.dt.bfloat16
    f8 = mybir.dt.float8e3
    i32 = mybir.dt.int32
    nc = bass.Bass()

    TOT = NT * FREE
    xn = nc.declare_dram_parameter("xn", [P, TOT], f8, isOutput=False)
    out = nc.declare_dram_parameter("out", [P, 1], f32, isOutput=True)

    AT = mybir.ActivationFunctionType
    add, mult = mybir.AluOpType.add, mybir.AluOpType.mult

    with tile.TileContext(nc) as tc, ExitStack() as ctx:
        cpool = ctx.enter_context(tc.tile_pool(name="const", bufs=1))
        xpool = ctx.enter_context(tc.tile_pool(name="xq", bufs=xbufs))
        epool = ctx.enter_context(tc.tile_pool(name="e", bufs=ebufs))
        tpool = ctx.enter_context(tc.tile_pool(name="t", bufs=3))
        spool = ctx.enter_context(tc.tile_pool(name="scr", bufs=2))

        cols = cpool.tile([P, max(reps, 2)], f32, tag="cols")
        dcol = cpool.tile([P, 1], f32, tag="dcol")
        nc.vector.memset(dcol[:], LN_D)
        dbc = dcol[:].broadcast_to([P, FREE])

        for rep in range(reps):
            # 3 pair-chunks [P, 2*FREE] + 1 single [P, FREE], each one DMA
            # with fully contiguous per-partition lines
            xcs = []
            for c in range(3):
                xc = xpool.tile([P, 2, FREE], f8, tag="xq")
                nc.sync.dma_start(
                    xc[:], xn[:, c * 2 * FREE:(c + 1) * 2 * FREE]
                    .rearrange("p (k f) -> p k f", k=2))
                xcs.append(xc)
            xc3 = xpool.tile([P, FREE], f8, tag="xq3")
            nc.sync.dma_start(xc3[:], xn[:, 6 * FREE:7 * FREE])
            if dma_only:
                continue

            es = []
            for c in range(3):
                e = epool.tile([P, 2, FREE], bf16, tag="e")
                nc.scalar.activation(e[:], xcs[c][:], AT.Sigmoid)
                es.append(e)
            e3 = epool.tile([P, FREE], bf16, tag="e3")
            nc.scalar.activation(e3[:], xc3[:], AT.Sigmoid)
            if drop_dve:
                continue

            t = tpool.tile([P, FREE], bf16, tag="t")
            nc.vector.tensor_mul(t[:], es[0][:, 0], es[0][:, 1])
            nc.vector.tensor_mul(t[:], t[:], es[1][:, 0])
            nc.vector.tensor_mul(t[:], t[:], es[1][:, 1])
            nc.vector.tensor_mul(t[:], t[:], es[2][:, 0])
            nc.vector.tensor_mul(t[:], t[:], es[2][:, 1])
            tf = tpool.tile([P, FREE], f32, tag="tf")
            nc.vector.tensor_mul(tf[:], t[:], e3[:])
            if drop_ln:
                continue

            scr = spool.tile([P, FREE], f32, tag="lnscr")
            nc.vector.scalar_tensor_tensor(
                scr[:], tf[:].bitcast(i32), LN_C, dbc,
                op0=mult, op1=add,
                accum_out=cols[:, rep:rep + 1])

        if dma_only or drop_dve or drop_ln:
            zcol = cpool.tile([P, 1], f32, tag="z")
            nc.vector.memset(zcol[:], 0.0)
            nc.sync.dma_start(out[:], zcol[:])
        else:
            red = cpool.tile([P, 1], f32, tag="red")
            nc.vector.tensor_reduce(red[:], cols[:], axis=mybir.AxisListType.X,
                                    op=mybir.AluOpType.add)
            nc.sync.dma_start(out[:], red[:])

    _split_excess_waits(nc, mybir, limit=1)
    return nc


def _split_excess_waits(nc, mybir, limit=1):
    """walrus core_v2/v3 codegen allows only `limit` fused sem waits per
    instruction; hoist the excess into standalone EventSemaphore waits."""
    fn = nc.m.functions[0]
    for blk in fn.blocks:
        out_instrs = []
        for inst in blk.instructions:
            si = getattr(inst, "sync_info", None)
            waits = list(si.on_wait) if si is not None and si.on_wait else []
            if len(waits) > limit:
                extra, keep = waits[:-limit], waits[-limit:]
                for i in range(0, len(extra), limit):
                    w = mybir.InstEventSemaphore(
                        name=f"{inst.name}_xw{i}", ins=[], outs=[]
                    )
                    w.engine = inst.engine
                    w.sync_info = mybir.SyncInfo(
                        on_wait=extra[i:i + limit], on_update=[]
                    )
                    nc.register_instruction(w)
                    out_instrs.append(w)
                si.on_wait = keep
            out_instrs.append(inst)
        if len(out_instrs) != len(blk.instructions):
            del blk.instructions[:]
            blk.instructions.extend(out_instrs)


def _host_prep(depth_gt, depth):
    """Quantize -x to fp8_e3m4 with the mask baked in; pack per core as
    [128, 7*2112] with partition lines contiguous in DRAM."""
    import ml_dtypes
    f8 = ml_dtypes.float8_e3m4
    depth_gt = np.asarray(depth_gt, dtype=np.float32)
    depth = np.asarray(depth, dtype=np.float32)
    assert depth_gt.shape == (B, N, H, W)
    assert depth.shape == (B, N * D, H, W)

    xq = (-depth).astype(f8)  # (B, N*D, H, W)
    inv = depth_gt == 0.0
    xq_v = xq.reshape(B, N, D, H, W)
    xq_v[np.broadcast_to(inv[:, :, None], (B, N, D, H, W))] = f8(MASK_VAL)

    in_maps = []
    for c in range(M):
        h0 = c * HSH
        # partition p = dd*HSH + hp ; free = t*FREE + (b*N+n)*W + w
        xc = xq_v[:, :, :, h0:h0 + HSH, :]          # (B,N,D,HSH,W)
        xc = xc.reshape(B, N, NT, DD, HSH, W)
        xc = xc.transpose(3, 4, 2, 0, 1, 5)          # (DD,HSH,NT,B,N,W)
        in_maps.append({"xn": np.ascontiguousarray(xc).reshape(P, NT * FREE)})
    return in_maps


def kernel(depth_gt, depth):
    from concourse.bass_utils import run_bass_kernel_spmd

    if "nc" not in _CACHE:
        _CACHE["nc"] = _build_bass()
    nc = _CACHE["nc"]

    depth_gt = np.asarray(depth_gt, dtype=np.float32)
    depth = np.asarray(depth, dtype=np.float32)
    in_maps = _host_prep(depth_gt, depth)
    res = run_bass_kernel_spmd(nc, in_maps, list(range(M)))
    a_dev = float(np.sum([r["out"].astype(np.float64).sum()
                          for r in res.results]))
    # masked columns read ln(1.0) = +kappa*ln2 each; subtract exactly
    n_masked_cols = int(np.count_nonzero(depth_gt == 0.0)) * DD
    a_total = -(a_dev - n_masked_cols * KAPPA * LN2)

    # one-hot gather term on host: touches only the ~135K indexed elements
    u = (depth_gt - np.float32(2.0)) * np.float32(2.0)
    idx = np.clip(np.floor(u), 0.0, float(D)).astype(np.int64)
    sel = (depth_gt != 0.0) & (idx < D)
    bb, nn, hh, ww = np.nonzero(sel)
    x5 = depth.reshape(B, N, D, H, W)
    b_total = float(x5[bb, nn, idx[sel], hh, ww].astype(np.float64).sum())
    return np.float32(3.0 * (a_total - b_total) / NUMEL)


# revision 4
# speedup vs baseline: 1.0089x; 1.0089x over previous
"""Trainium2 Bass kernel for DepthLossForImgBEV (weighted one-hot depth BCE).

Math: with x = raw logits (B,N,D,H,W), gt = depth_gt (B,N,H,W):
  loss = 3.0 * [ sum_{valid px, d} softplus(x) - sum_{valid px} x[idx] ] / numel
and the device computes sum softplus via the sigmoid-product identity
  sum_d softplus(x_d) = -ln prod_d sigmoid(-x_d),
so the only transcendental pass is ONE sigmoid over the logits; the
depth-bin reduction becomes a pure multiply chain and the final ln is a
1-instruction bit-trick on DVE. The ACT engine never switches tables.

Device pipeline per core (shard = 8 of 64 H-rows; partitions = 16 depth
bins x 8 h-rows; free = 7 t-tiles x (12 cameras x 176 w)):
  - input xn = fp8_e3m4(-x), host-packed [128, 7*2112] fully contiguous
    per partition; invalid pixels baked to +15.5 (sigmoid -> 1.0 in bf16),
    which removes the mask tensor and mask-add pass entirely.
  - ACT: sigmoid(xn) per chunk, fp8-in -> bf16-out (4 instrs, one
    ACT_TABLE_LOAD for the whole NEFF).
  - DVE: t = prod_t sigmoid(-x_t): 5 in-place bf16 muls (2-byte 2x mode)
    + 1 f32-out mul over the head columns; the LAST Z=1024 columns of
    tile 6 are offloaded to DVE entirely: ln sigma(v) = -ln(1+e^{-v})
    via Schraudolph exp (stt f32->i32 convert), add-1, and the same
    bit-trick ln, accumulated into a second column set and subtracted
    in the epilogue — trims the ACT-bound sigmoid pass by ~7%.
  - DVE: ln(t) ~= (bitcast_i32(t) * ln2/2^23) - 127*ln2 + kappa*ln2, one
    scalar_tensor_tensor with accum_out. kappa = mean(log2(1+f)-f) zero-
    means the mantissa-interp error for log-uniform mantissae; masked
    columns (t == 1.0 exactly) each read +kappa*ln2, which the host
    subtracts exactly (count = 16 * #invalid pixels).
Host: a_total = -(sum(out) - n_masked_cols*kappa*ln2); one-hot gather term
b_total by fancy-indexing exact f32 logits (0.4% of FLOPs); loss =
3*(a_total - b_total)/numel.

Measured on trn2 (8 cores, axon), interleaved same-round reps-slope
(reps=600 vs 1), median over rounds:
  - previous kernel (exp + pair-fold + ln):  25.3 us/pass
  - this kernel:                             11.4 us/pass  (2.2x)
Same-window ablations: DMA floor 3.7 us, sigmoid-only 15.3/16.3 of full,
+muls +0.4 us, +lnapprox +0.6 us -> ACT-sigmoid-bound; DVE and DMA fully
hidden. Engine rates calibrated on HW: ACT ~0.86-1.03 ns/el (fp8 input
fastest), DVE bf16 mul 0.46 (2x), stt 0.98 (no 2x), DMA ~0.27-0.35 ns/B.
Rel err vs fp32 jax reference: 5.7e-05 on HW (numpy bit-exact sim of the
pipeline predicts 5.8e-05; fp8-e3m4 quantization and bf16 product noise
are zero-mean and cancel over the 12.6M-element sum).

Notes:
  - walrus core_v2/v3 codegen accepts only ONE fused sem wait per
    instruction on this toolchain -> _split_excess_waits hoists extras
    into standalone EventSemaphore instructions.
  - softplus is not in this compiler's act tables (the set exists but maps
    no Softplus function); sigmoid+ln share no table set, hence the DVE
    bit-trick ln instead of ACT Ln.
"""
import numpy as np

B, N, D, H, W = 2, 6, 112, 64, 176
M = 8  # cores
HSH = H // M  # 8 h-rows per core
DD = 16  # depth bins per partition block
NT = D // DD  # 7 t-tiles
BN = B * N  # 12
P = 128
FREE = BN * W  # 2112
NUMEL = B * N * D * H * W
MASK_VAL = 15.5  # sigmoid(15.5) = 1 - 1.9e-7 -> rounds to 1.0 in bf16

LN2 = 0.6931471805599453
# kappa (bits): zero-means f - log2(1+f) over f~U[0,1]
KAPPA = (2.0 * LN2 - 1.0) / LN2 - 0.5  # 0.057304959...
LN_C = LN2 / (1 << 23)
LN_D = (-127.0 + KAPPA) * LN2

_CACHE = {}


Z = 1024  # tile-6 tail columns whose softplus runs on DVE instead of ACT


def _build_bass(reps=1, dma_only=False, drop_dve=False, drop_ln=False,
                xbufs=5, ebufs=4):
    from contextlib import ExitStack

    import concourse.bass as bass
    import concourse.mybir as mybir
    import concourse.tile as tile

    f32 = mybir.dt.float32
    bf16 = mybir.dt.bfloat16
    f8 = mybir.dt.float8e3
    i32 = mybir.dt.int32
    nc = bass.Bass()

    TOT = NT * FREE
    xn = nc.declare_dram_parameter("xn", [P, TOT], f8, isOutput=False)
    out = nc.declare_dram_parameter("out", [P, 1], f32, isOutput=True)

    AT = mybir.ActivationFunctionType
    add, mult = mybir.AluOpType.add, mybir.AluOpType.mult

    with tile.TileContext(nc) as tc, ExitStack() as ctx:
        cpool = ctx.enter_context(tc.tile_pool(name="const", bufs=1))
        xpool = ctx.enter_context(tc.tile_pool(name="xq", bufs=xbufs))
        epool = ctx.enter_context(tc.tile_pool(name="e", bufs=ebufs))
        tpool = ctx.enter_context(tc.tile_pool(name="t", bufs=3))
        spool = ctx.enter_context(tc.tile_pool(name="scr", bufs=2))

        HD = FREE - Z
        EXP_A = float(1 << 23) / LN2
        EXP_B = 1065353216.0 - 482870.0  # Schraudolph, zero-mean c
        cols = cpool.tile([P, max(reps, 2)], f32, tag="cols")
        cols2 = cpool.tile([P, max(reps, 2)], f32, tag="cols2")
        dcol = cpool.tile([P, 1], f32, tag="dcol")
        nc.vector.memset(dcol[:], LN_D)
        dbc = dcol[:].broadcast_to([P, FREE])
        dbz = dcol[:].broadcast_to([P, Z])
        bcol = cpool.tile([P, 1], f32, tag="bcol")
        nc.vector.memset(bcol[:], EXP_B)
        bbz = bcol[:].broadcast_to([P, Z])

        for rep in range(reps):
            # 3 pair-chunks [P, 2*FREE] + 1 single [P, FREE], each one DMA
            # with fully contiguous per-partition lines
            xcs = []
            for c in range(3):
                xc = xpool.tile([P, 2, FREE], f8, tag="xq")
                nc.sync.dma_start(
                    xc[:], xn[:, c * 2 * FREE:(c + 1) * 2 * FREE]
                    .rearrange("p (k f) -> p k f", k=2))
                xcs.append(xc)
            xc3 = xpool.tile([P, FREE], f8, tag="xq3")
            nc.sync.dma_start(xc3[:], xn[:, 6 * FREE:7 * FREE])
            if dma_only:
                continue

            es = []
            for c in range(3):
                e = epool.tile([P, 2, FREE], bf16, tag="e")
                nc.scalar.activation(e[:], xcs[c][:], AT.Sigmoid)
                es.append(e)
            e3 = epool.tile([P, FREE], bf16, tag="e3")
            nc.scalar.activation(e3[:, 0:HD], xc3[:, 0:HD], AT.Sigmoid)
            if drop_dve:
                continue

            # DVE offload: ln(1+exp(-v)) for tile 6's tail columns
            y32 = spool.tile([P, Z], i32, tag="y32")
            nc.vector.scalar_tensor_tensor(
                y32[:], xc3[:, HD:FREE], -EXP_A, bbz, op0=mult, op1=add)
            wz = spool.tile([P, Z], f32, tag="wz")
            nc.vector.tensor_scalar_add(wz[:], y32[:].bitcast(f32), 1.0)
            scr2 = spool.tile([P, Z], f32, tag="scr2")
            nc.vector.scalar_tensor_tensor(
                scr2[:], wz[:].bitcast(i32), LN_C, dbz,
                op0=mult, op1=add, accum_out=cols2[:, rep:rep + 1])

            t = tpool.tile([P, FREE], bf16, tag="t")
            nc.vector.tensor_mul(t[:], es[0][:, 0], es[0][:, 1])
            nc.vector.tensor_mul(t[:], t[:], es[1][:, 0])
            nc.vector.tensor_mul(t[:], t[:], es[1][:, 1])
            nc.vector.tensor_mul(t[:], t[:], es[2][:, 0])
            nc.vector.tensor_mul(t[:], t[:], es[2][:, 1])
            tf = tpool.tile([P, FREE], f32, tag="tf")
            nc.vector.tensor_mul(tf[:, 0:HD], t[:, 0:HD], e3[:, 0:HD])
            nc.vector.tensor_copy(tf[:, HD:FREE], t[:, HD:FREE])
            if drop_ln:
                continue

            scr = spool.tile([P, FREE], f32, tag="lnscr")
            nc.vector.scalar_tensor_tensor(
                scr[:], tf[:].bitcast(i32), LN_C, dbc,
                op0=mult, op1=add,
                accum_out=cols[:, rep:rep + 1])

        if dma_only or drop_dve or drop_ln:
            zcol = cpool.tile([P, 1], f32, tag="z")
            nc.vector.memset(zcol[:], 0.0)
            nc.sync.dma_start(out[:], zcol[:])
        else:
            r1 = cpool.tile([P, 1], f32, tag="r1")
            nc.vector.tensor_reduce(r1[:], cols[:], axis=mybir.AxisListType.X,
                                    op=mybir.AluOpType.add)
            r2 = cpool.tile([P, 1], f32, tag="r2")
            nc.vector.tensor_reduce(r2[:], cols2[:],
                                    axis=mybir.AxisListType.X,
                                    op=mybir.AluOpType.add)
            red = cpool.tile([P, 1], f32, tag="red")
            nc.vector.tensor_sub(red[:], r1[:], r2[:])
            nc.sync.dma_start(out[:], red[:])

    _split_excess_waits(nc, mybir, limit=1)
    return nc


def _split_excess_waits(nc, mybir, limit=1):
    """walrus core_v2/v3 codegen allows only `limit` fused sem waits per
    instruction; hoist the excess into standalone EventSemaphore waits."""
    fn = nc.m.functions[0]
    for blk in fn.blocks:
        out_instrs = []
        for inst in blk.instructions:
            si = getattr(inst, "sync_info", None)
            waits = list(si.on_wait) if si is not None and si.on_wait else []
            if len(waits) > limit:
                extra, keep = waits[:-limit], waits[-limit:]
                for i in range(0, len(extra), limit):
                    w = mybir.InstEventSemaphore(
                        name=f"{inst.name}_xw{i}", ins=[], outs=[]
                    )
                    w.engine = inst.engine
                    w.sync_info = mybir.SyncInfo(
                        on_wait=extra[i:i + limit], on_update=[]
                    )
                    nc.register_instruction(w)
                    out_instrs.append(w)
                si.on_wait = keep
            out_instrs.append(inst)
        if len(out_instrs) != len(blk.instructions):
            del blk.instructions[:]
            blk.instructions.extend(out_instrs)


def _host_prep(depth_gt, depth):
    """Quantize -x to fp8_e3m4 with the mask baked in; pack per core as
    [128, 7*2112] with partition lines contiguous in DRAM."""
    import ml_dtypes
    f8 = ml_dtypes.float8_e3m4
    depth_gt = np.asarray(depth_gt, dtype=np.float32)
    depth = np.asarray(depth, dtype=np.float32)
    assert depth_gt.shape == (B, N, H, W)
    assert depth.shape == (B, N * D, H, W)

    xq = (-depth).astype(f8)  # (B, N*D, H, W)
    inv = depth_gt == 0.0
    xq_v = xq.reshape(B, N, D, H, W)
    xq_v[np.broadcast_to(inv[:, :, None], (B, N, D, H, W))] = f8(MASK_VAL)

    in_maps = []
    for c in range(M):
        h0 = c * HSH
        # partition p = dd*HSH + hp ; free = t*FREE + (b*N+n)*W + w
        xc = xq_v[:, :, :, h0:h0 + HSH, :]          # (B,N,D,HSH,W)
        xc = xc.reshape(B, N, NT, DD, HSH, W)
        xc = xc.transpose(3, 4, 2, 0, 1, 5)          # (DD,HSH,NT,B,N,W)
        in_maps.append({"xn": np.ascontiguousarray(xc).reshape(P, NT * FREE)})
    return in_maps


def kernel(depth_gt, depth):
    from concourse.bass_utils import run_bass_kernel_spmd

    if "nc" not in _CACHE:
        _CACHE["nc"] = _build_bass()
    nc = _CACHE["nc"]

    depth_gt = np.asarray(depth_gt, dtype=np.float32)
    depth = np.asarray(depth, dtype=np.float32)
    in_maps = _host_prep(depth_gt, depth)
    res = run_bass_kernel_spmd(nc, in_maps, list(range(M)))
    a_dev = float(np.sum([r["out"].astype(np.float64).sum()
                          for r in res.results]))
    # masked columns read ln(1.0) = +kappa*ln2 each; subtract exactly.
    # Offloaded (tile-6 tail) masked elements sit in cols2, which the
    # device SUBTRACTS, so their kappa cancels against the cols side.
    inv = depth_gt == 0.0
    n_masked_cols = int(np.count_nonzero(inv)) * DD
    bb_, nn_, ww_ = np.meshgrid(np.arange(B), np.arange(N), np.arange(W),
                                indexing="ij")
    tail = ((bb_ * N + nn_) * W + ww_) >= FREE - Z
    n_masked_tail = int(np.count_nonzero(inv & tail[:, :, None, :])) * DD
    a_total = -(a_dev - (n_masked_cols - n_masked_tail) * KAPPA * LN2)

    # one-hot gather term on host: touches only the ~135K indexed elements
    u = (depth_gt - np.float32(2.0)) * np.float32(2.0)
    idx = np.clip(np.floor(u), 0.0, float(D)).astype(np.int64)
    sel = (depth_gt != 0.0) & (idx < D)
    bb, nn, hh, ww = np.nonzero(sel)
    x5 = depth.reshape(B, N, D, H, W)
    b_total = float(x5[bb, nn, idx[sel], hh, ww].astype(np.float64).sum())
    return np.float32(3.0 * (a_total - b_total) / NUMEL)
